# revision 27
# baseline (speedup 1.0000x reference)
"""GCN feature extractor on 8 Trainium2 NeuronCores.

Row-parallel sharding over the dense normalized adjacency A (symmetric).
Each core c owns a 1024-node block and computes, entirely on-device:

  Y    = X^T @ A[:, blk]                    ([FIN, BLK])   K=N matmul
  M    = Ppool @ A[:, blk]                  ([B, BLK])     same stream!
  H1^T = relu(W1^T @ Y + b1 (x) rowsums)    ([HID, BLK])
  Z    = H1 @ W2 + b2                       ([BLK, OUT])
  out  = Z^T @ M^T                          ([OUT, B])     partial

The host sums the 8 [OUT, B] partials (the pooling "all-reduce") and
transposes. Two reassociations make this fast:
  * layer 1 runs as (A @ X) @ W1 (F_IN < HID halves the wide matmul),
  * pooling runs as (Ppool @ A) @ Z instead of Ppool @ (A @ Z), so M
    rides the SAME A[:, blk] chunks phase 2a already streams — the
    16MB A[blk, :] stream and its whole phase disappear.
Per-chunk DMA is one transfer: axk[k] = [X_k | Acol_k | Ppool^T_k].
1/count is folded into Ppool host-side; b1's adjacency product folds in
as a zero-padded rank-1 matmul with host-precomputed rowsums(A).
"""

import numpy as np
import ml_dtypes

import concourse.bass as bass
import concourse.mybir as mybir
import concourse.tile as tile
from concourse.masks import make_identity
from concourse.vector_clock import ScopedClock
from concourse.bass_utils import run_bass_kernel_spmd

N, FIN, HID, OUT, B, NCORES = 8192, 128, 256, 128, 64, 8
BLK = N // NCORES  # 1024
P = 128
AXKW = FIN + BLK + B  # 1216 columns per streamed chunk

# Matmul operand dtype for the big streams. bf16 keeps both the PE and
# HBM sides at the roofline ridge; PSUM accumulation is always fp32, and
# the small M/Z tail matmuls run in fp32.
DT = mybir.dt.bfloat16
NP_DT = ml_dtypes.bfloat16


def _legalize_waits(nc, max_waits=1):
    """This walrus build only accepts a single semaphore wait per
    instruction; Tile attaches as many as the dependence structure
    needs. Hoist excess waits onto pure-wait EventSemaphore
    instructions (what wait_ge emits) inserted just before the owner."""

    def fix_block(blk):
        for sub in getattr(blk, "blocks", None) or []:
            fix_block(sub)
        insts = list(blk.instructions)
        out = []
        changed = False
        for inst in insts:
            si = getattr(inst, "sync_info", None)
            waits = list(si.on_wait) if si is not None else []
            if len(waits) > max_waits:
                changed = True
                inst.sync_info = mybir.SyncInfo(
                    on_wait=waits[-max_waits:], on_update=list(si.on_update)
                )
                for j, w in enumerate(waits[:-max_waits]):
                    out.append(
                        mybir.InstEventSemaphore(
                            name=f"{inst.name}-hw{j}",
                            engine=inst.engine,
                            ins=[],
                            outs=[],
                            sync_info=mybir.SyncInfo(on_wait=[w], on_update=[]),
                        )
                    )
            out.append(inst)
        if changed:
            blk.instructions = out

    for fn in nc.m.functions:
        for blk in fn.blocks:
            fix_block(blk)


class _TileContext(tile.TileContext):
    def _drain_and_barrier(self, tick_clock, wait_clock):
        nc = self.nc
        drain_inst = nc.sync.drain()
        wait_clock.add_sem_waits(
            drain_inst.ins, ScopedClock({None: tick_clock.global_clock})
        )
        si = drain_inst.ins.sync_info
        waits = list(si.on_wait) if si is not None else []
        if len(waits) > 1:
            drain_inst.ins.sync_info = mybir.SyncInfo(
                on_wait=waits[:1], on_update=list(si.on_update)
            )
            for w in waits[1:]:
                extra = nc.sync.drain()
                extra.ins.sync_info = mybir.SyncInfo(on_wait=[w], on_update=[])
        nc.all_engine_barrier()
        popped = nc._tile_sem_poison_stack.pop()
        assert popped is self._sem_poison
        assert self.sems is not None
        nc.clear_and_free_semaphores(list(self.sems.allocated().values()))
        nc.all_engine_barrier()


def build_program():
    nc = bass.Bass()
    f32 = mybir.dt.float32

    axk_d = nc.dram_tensor("axk", [64, P, AXKW], DT, kind="ExternalInput")
    w1_d = nc.dram_tensor("w1", [P, HID], DT, kind="ExternalInput")
    # b1pad/rpad: row 0 carries b1 / rowsums(A)_blk, rows 1..127 zero —
    # the b1 (x) r rank-1 update runs as a full K=128 matmul (K<128
    # matmuls are broken in this stack).
    b1row_d = nc.dram_tensor("b1row", [P, HID], DT, kind="ExternalInput")
    rrow_d = nc.dram_tensor("rrow", [P, BLK], DT, kind="ExternalInput")
    w2_d = nc.dram_tensor("w2", [2, P, OUT], DT, kind="ExternalInput")
    b2r_d = nc.dram_tensor("b2r", [P, OUT], f32, kind="ExternalInput")
    cinv_d = nc.dram_tensor("cinv", [P, B], f32, kind="ExternalInput")
    out_d = nc.dram_tensor("outp", [P, B], f32, kind="ExternalOutput")

    KC = N // P  # 64 contraction chunks for the adjacency matmuls

    with _TileContext(nc) as tc:
        with (
            tc.tile_pool(name="const", bufs=1) as cpool,
            tc.tile_pool(name="h1t", bufs=1) as hpool,
            tc.tile_pool(name="z", bufs=1) as zpool,
            tc.tile_pool(name="ysb", bufs=1) as ypool,
            tc.tile_pool(name="msb", bufs=1) as mpool,
        ):
            # Constants on the scalar DGE queue so the sync queue starts
            # streaming axk immediately.
            w1_sb = cpool.tile([P, HID], DT)
            nc.scalar.dma_start(w1_sb[:], w1_d[:])
            b1row_sb = cpool.tile([P, HID], DT)
            nc.scalar.dma_start(b1row_sb[:], b1row_d[:])
            rrow_sb = cpool.tile([P, BLK], DT)
            nc.scalar.dma_start(rrow_sb[:], rrow_d[:])
            w2_sb = [
                cpool.tile([P, OUT], DT, tag=f"w2_{k}", name=f"w2_{k}")
                for k in range(2)
            ]
            for k in range(2):
                nc.scalar.dma_start(w2_sb[k][:], w2_d[k])
            b2r_sb = cpool.tile([P, OUT], f32)
            nc.scalar.dma_start(b2r_sb[:], b2r_d[:])
            cinv_sb = cpool.tile([P, B], f32)
            nc.scalar.dma_start(cinv_sb[:], cinv_d[:])
            ident_sb = cpool.tile([P, P], f32)
            make_identity(nc, ident_sb[:])

            h1t_sb = [
                hpool.tile([P, BLK], DT, tag=f"h1t_{m}", name=f"h1t_{m}")
                for m in range(2)
            ]
            z_sb = [
                zpool.tile([P, OUT], f32, tag=f"z_{m}", name=f"z_{m}")
                for m in range(8)
            ]
            y_sb = ypool.tile([P, BLK], DT)
            # M stacked: partitions 0:64 hold M[:, 0:512], 64:128 hold
            # M[:, 512:1024] (from the col-tiled pair below).
            m_sb = mpool.tile([P, 512], f32)
            mt_sb = [
                mpool.tile([P, B], f32, tag=f"mt_{m}", name=f"mt_{m}")
                for m in range(8)
            ]

            with (
                tc.tile_pool(name="acol", bufs=20) as apool,
                tc.tile_pool(name="psum_y", bufs=1, space="PSUM") as pypool,
                tc.tile_pool(name="psum_m", bufs=1, space="PSUM") as pmpool,
            ):
                # Phase 2a: Y = X^T @ Acol and M = Ppool @ Acol, K = N
                # accumulation over one fused [X | Acol | Ppool^T] stream.
                psy = [
                    pypool.tile([P, 512], f32, tag=f"psy_{nn}", name=f"psy_{nn}")
                    for nn in range(2)
                ]
                # One bank for both M halves: the two M=64 matmuls run
                # as a concurrent col-tiled pair into partitions 0:64 and
                # 64:128.
                psm = pmpool.tile([P, 512], f32, tag="psm", name="psm")
                for k in range(KC):
                    ac = apool.tile([P, AXKW], DT, tag="acol", name=f"acol_{k}")
                    nc.sync.dma_start(ac[:], axk_d[k])
                    xs = ac[:, 0:FIN]
                    ps = ac[:, FIN + BLK : AXKW]
                    for nn in range(2):
                        nc.tensor.matmul(
                            psy[nn][:], xs,
                            ac[:, FIN + nn * 512 : FIN + (nn + 1) * 512],
                            start=(k == 0), stop=(k == KC - 1),
                        )
                    for nn in range(2):
                        nc.tensor.matmul(
                            psm[nn * B : (nn + 1) * B, :], ps,
                            ac[:, FIN + nn * 512 : FIN + (nn + 1) * 512],
                            start=(k == 0), stop=(k == KC - 1),
                            tile_position=(0, nn * B),
                        )
                for nn in range(2):
                    nc.scalar.activation(
                        y_sb[:, nn * 512 : (nn + 1) * 512],
                        psy[nn][:],
                        mybir.ActivationFunctionType.Copy,
                    )
                nc.scalar.activation(
                    m_sb[:], psm[:], mybir.ActivationFunctionType.Copy
                )


            with (
                tc.tile_pool(name="psum_h", bufs=1, space="PSUM") as phpool,
                tc.tile_pool(name="psum_t", bufs=2, space="PSUM") as ptpool,
                tc.tile_pool(name="psum_z", bufs=2, space="PSUM") as pzpool,
            ):
                # Phase 2b: H1^T = relu(W1^T @ Y + b1 (x) rowsums(A)_blk)
                for mc in range(2):
                    for nn in range(2):
                        psh = phpool.tile(
                            [P, 512], f32, tag=f"psh_{mc}_{nn}",
                            name=f"psh_{mc}_{nn}",
                        )
                        nc.tensor.matmul(
                            psh[:],
                            w1_sb[:, mc * P : (mc + 1) * P],
                            y_sb[:, nn * 512 : (nn + 1) * 512],
                            start=True,
                            stop=False,
                        )
                        nc.tensor.matmul(
                            psh[:],
                            b1row_sb[:, mc * P : (mc + 1) * P],
                            rrow_sb[:, nn * 512 : (nn + 1) * 512],
                            start=False,
                            stop=True,
                        )
                        nc.scalar.activation(
                            h1t_sb[mc][:, nn * 512 : (nn + 1) * 512],
                            psh[:],
                            mybir.ActivationFunctionType.Relu,
                        )

                # M^T chunks via PE transpose: each [128,128] slice of
                # the stacked M holds graphs x (two node ranges); its
                # transpose yields M^T for node chunks c and c+4.
                for c in range(4):
                    pst = ptpool.tile([P, P], f32, tag="pst", name=f"pst_{c}")
                    nc.tensor.transpose(
                        pst[:], m_sb[:, c * P : (c + 1) * P], ident_sb[:]
                    )
                    nc.vector.tensor_copy(mt_sb[c][:], pst[:, 0:B])
                    nc.vector.tensor_copy(mt_sb[c + 4][:], pst[:, B:P])

                # Phase 3: Z = H1 @ W2 + b2  (fp32, nodes-on-partitions)
                for mz in range(8):
                    ps = pzpool.tile([P, OUT], f32, tag="psz", name=f"psz_{mz}")
                    for kz in range(2):
                        nc.tensor.matmul(
                            ps[:],
                            h1t_sb[kz][:, mz * P : (mz + 1) * P],
                            w2_sb[kz][:],
                            start=(kz == 0),
                            stop=(kz == 1),
                        )
                    nc.vector.tensor_tensor(
                        z_sb[mz][:], ps[:], b2r_sb[:], mybir.AluOpType.add
                    )

            with tc.tile_pool(name="psum_o", bufs=1, space="PSUM") as popool:
                # Output: pooled^T partial = Z^T @ M^T  ([OUT, B], fp32)
                pso = popool.tile([P, B], f32)
                for kz in range(8):
                    nc.tensor.matmul(
                        pso[:],
                        z_sb[kz][:],
                        mt_sb[kz][:],
                        start=(kz == 0),
                        stop=(kz == 7),
                    )
                osb = mpool.tile([P, B], f32, name="osb")
                nc.vector.tensor_tensor(
                    osb[:], pso[:], cinv_sb[:], mybir.AluOpType.mult
                )
                nc.sync.dma_start(out_d[:], osb[:])

    _legalize_waits(nc)
    return nc


def _host_prep(node_features, W1, b1, W2, b2, edge_index, batch, num_graphs):
    x = np.asarray(node_features, dtype=np.float32)
    W1 = np.asarray(W1, dtype=np.float32)
    b1 = np.asarray(b1, dtype=np.float32)
    W2 = np.asarray(W2, dtype=np.float32)
    b2 = np.asarray(b2, dtype=np.float32)
    ei = np.asarray(edge_index).astype(np.int64)
    batch = np.asarray(batch).astype(np.int64)
    nb = int(num_graphs)

    n = x.shape[0]
    # Dense normalized adjacency, matching the reference exactly:
    # set (dedup) both directions, then add I (so a self-edge gives 2.0).
    A = np.zeros((n, n), dtype=np.float32)
    A[ei[0], ei[1]] = 1.0
    A[ei[1], ei[0]] = 1.0
    A[np.arange(n), np.arange(n)] += 1.0
    deg = A.sum(axis=1, dtype=np.float32)
    dis = np.where(deg > 0, 1.0 / np.sqrt(deg, dtype=np.float32), 0.0).astype(
        np.float32
    )
    A *= dis[:, None]
    A *= dis[None, :]
    rs = A.sum(axis=1, dtype=np.float32)  # normalized-adjacency rowsums

    counts = np.bincount(batch, minlength=nb).astype(np.int64)
    cinv = (1.0 / np.maximum(counts, 1)).astype(np.float32)
    # Mean-pooling matrix transpose, exact 0/1 entries in bf16; the
    # 1/count scale is applied on-device in f32.
    ppool = np.zeros((n, B), dtype=np.float32)
    ppool[np.arange(n), batch] = 1.0
    cinvr = np.broadcast_to(cinv, (P, B)).copy()

    w1t = W1.astype(NP_DT)  # [FIN, HID]
    b1pad = np.zeros((P, HID), dtype=np.float32)
    b1pad[0] = b1
    w2t = W2.reshape(2, P, OUT).astype(NP_DT)
    b2r = np.broadcast_to(b2, (P, OUT)).copy()

    xr = x.reshape(64, P, FIN)
    pr = ppool.reshape(64, P, B)

    in_maps = []
    for c in range(NCORES):
        lo, hi = c * BLK, (c + 1) * BLK
        rpad = np.zeros((P, BLK), dtype=np.float32)
        rpad[0] = rs[lo:hi]
        axk = np.concatenate(
            [xr, A[:, lo:hi].reshape(64, P, BLK), pr], axis=2
        ).astype(NP_DT)
        in_maps.append(
            {
                "axk": axk,
                "w1": w1t,
                "b1row": b1pad.astype(NP_DT),
                "rrow": rpad.astype(NP_DT),
                "w2": w2t,
                "b2r": b2r,
                "cinv": cinvr,
            }
        )
    return in_maps, [], nb


def kernel(
    node_features, W1, b1, W2, b2, edge_index, batch, num_graphs, **_unused
):
    in_maps, _, nb = _host_prep(
        node_features, W1, b1, W2, b2, edge_index, batch, num_graphs
    )
    nc = build_program()
    try:
        res = run_bass_kernel_spmd(nc, in_maps, core_ids=list(range(NCORES)))
    except Exception:
        # Transient NRT exec-unit wedges recover on retry.
        res = run_bass_kernel_spmd(nc, in_maps, core_ids=list(range(NCORES)))
    acc = np.zeros((P, B), dtype=np.float32)
    for r in res.results:
        acc += r["outp"]
    return np.ascontiguousarray(acc.T[:nb]).astype(np.float32)
